# revision 20
# baseline (speedup 1.0000x reference)
"""Trainium2 Bass kernel for nn_CompressedCausalAttention.

Sharding: 8 cores = 2 batches x 4 head-groups (2 heads each).
Per-core dataflow (chan-major "T" layouts are (channel partition, seq free)):
  phase 2 (per s-window, woven into the attention loop of the previous
           window): xpeT = x+pe (bf16 DVE add), qT/kT chan-major (bias via
           gpsimd), v seq-major into vsb chunks laid out
           [ones(64) | v_h0(64) | v_h1(64) | ones(64)] so each head's AV
           matmul replicates the softmax denominator into 64 PSUM rows for
           free.
  phase 3: flash attention, scores (t-part, s-free), no max subtraction
           (scores bounded), causal mask = 0/1 multiply on gpsimd after exp,
           denominator inverted with reciprocal_approx_fast.
  phase 4: partial output projection outpT = Wc_mine^T-slice @ attnTn, woven
           into the next window's attention loop.
Host: shards inputs (incl. transposes, bf16 cast), sums the 4 per-batch
partials, adds bc (v-bias folded through the output projection).
"""

import numpy as np
import ml_dtypes

S, B, C, H = 2048, 2, 512, 8
CC = C // H            # 64
HPC = 2                # heads per core
NCORE = 8
SW = 512               # s window (free dim of score tiles)
TCH = 128              # t chunk (partition dim of score tiles)
TEMP = 1.0 / 8.0       # 1/sqrt(CC)

_CACHE = {}


def _build_bass(debug=False):
    import concourse.bass as bass
    import concourse.mybir as mybir
    import concourse.tile as tile
    from concourse import bacc

    f32 = mybir.dt.float32
    bf16 = mybir.dt.bfloat16

    nc = bacc.Bacc("TRN2", target_bir_lowering=False)
    xpe_d = nc.declare_dram_parameter("xpe", [4, 128, 4, 2, SW], bf16, isOutput=False)
    w3t = nc.declare_dram_parameter("w3t", [128, 4, 384], bf16, isOutput=False)
    b3 = nc.declare_dram_parameter("b3", [128, 2], f32, isOutput=False)
    wct = nc.declare_dram_parameter("wct", [128, C], bf16, isOutput=False)
    mask01 = nc.declare_dram_parameter("mask01", [128, 2, 128], bf16, isOutput=False)
    outp = nc.declare_dram_parameter("outp", [C, S], bf16, isOutput=True)
    if debug:
        dq = nc.declare_dram_parameter("dq", [128, S], bf16, isOutput=True)
        dk = nc.declare_dram_parameter("dk", [128, S], bf16, isOutput=True)
        dv = nc.declare_dram_parameter("dv", [128, 16, 4, CC], bf16, isOutput=True)
        dpb = nc.declare_dram_parameter("dpb", [128, 2, SW], bf16, isOutput=True)
        dav = nc.declare_dram_parameter("dav", [128, 2, SW], f32, isOutput=True)
        datn = nc.declare_dram_parameter("datn", [128, 4, SW], bf16, isOutput=True)

    Exp = mybir.ActivationFunctionType.Exp

    with tile.TileContext(nc) as tc:
        with (
            tc.tile_pool(name="singles", bufs=1) as singles,
            tc.tile_pool(name="pbp", bufs=4) as pbp,
            tc.tile_pool(name="atp", bufs=2) as atp,
            tc.tile_pool(name="rbp", bufs=2) as rbp,
            tc.tile_pool(name="osp", bufs=4) as osp,
            tc.tile_pool(name="psS", bufs=2, space="PSUM") as psS,
            tc.tile_pool(name="psV", bufs=1, space="PSUM") as psV,
            tc.tile_pool(name="psO", bufs=2, space="PSUM") as psO,
        ):
            # ---- constants + window-0/1 inputs (DMA'd first) ----
            w3t_sb = singles.tile([128, 4, 384], bf16, tag="w3t")
            nc.sync.dma_start(out=w3t_sb, in_=w3t[:, :, :])
            # x/pe interleaved per window: [128, w, k, {x,pe}, s']
            xpw = singles.tile([128, 4, 4, 2, SW], bf16, tag="xpw")
            for k in range(4):
                nc.sync.dma_start(out=xpw[:, 0, k], in_=xpe_d[0, :, k])
            b3_sb = singles.tile([128, 2], f32, tag="b3")
            nc.sync.dma_start(out=b3_sb, in_=b3[:, :])
            nc.sync.dma_start(out=xpw[:, 1], in_=xpe_d[1])
            wct_sb = singles.tile([128, C], bf16, tag="wct")
            nc.sync.dma_start(out=wct_sb, in_=wct[:, :])
            mask_sb = singles.tile([128, 2, 128], bf16, tag="mask")
            nc.sync.dma_start(out=mask_sb, in_=mask01[:, :, :])

            xpeT = [singles.tile([128, S], bf16, tag=f"xpeT{k}", name=f"xpeT{k}")
                    for k in range(4)]
            qT = singles.tile([128, S], bf16, tag="qT")
            kT = singles.tile([128, S], bf16, tag="kT")
            # v seq-major: chunk layout [ones | v_h0 | ones | v_h1]; head h's
            # AV stationary is cols [128*h : 128*h+128] = [ones | v_h] -> AV
            # output rows 0:CC hold the softmax denominator replicated (base
            # partition 0, required by the custom-DVE reciprocal), rows
            # CC:2CC hold the values.
            vsb = singles.tile([128, 16, 4, CC], bf16, tag="vsb")
            nc.vector.memset(vsb[:, :, 0, :], 1.0)
            nc.vector.memset(vsb[:, :, 2, :], 1.0)

            def emit_phase2_slices(w):
                """Emit-closures for phase-2 of window w (q/k/v projections)."""
                sl = slice(w * SW, (w + 1) * SW)

                def adds():
                    def go():
                        if 1 <= w < 3:
                            # this slice runs during window w-1; kick off the
                            # DMA for window w+1 (w0/w1 are loaded up front)
                            nc.sync.dma_start(out=xpw[:, w + 1],
                                              in_=xpe_d[w + 1])
                        eng = nc.vector if w == 0 else nc.gpsimd
                        for k in range(4):
                            eng.tensor_add(
                                out=xpeT[k][:, sl],
                                in0=xpw[:, w, k, 0, :], in1=xpw[:, w, k, 1, :])
                    return go

                def qk(blk, dst):
                    def go():
                        ps = psO.tile([128, SW], f32, tag="o")
                        for k in range(4):
                            nc.tensor.matmul(
                                ps,
                                lhsT=w3t_sb[:, k, blk * 128:(blk + 1) * 128],
                                rhs=xpeT[k][:, sl],
                                start=(k == 0), stop=(k == 3),
                            )
                        nc.vector.tensor_scalar_add(
                            out=dst[:, sl], in0=ps,
                            scalar1=b3_sb[:, blk:blk + 1])
                    return go

                def vgrp(st0):
                    def go():
                        ps = psO.tile([128, 2, 2, CC], f32, tag="o")
                        for p_ in range(2):
                            for k in range(4):
                                nc.tensor.matmul(
                                    ps[:, p_],
                                    lhsT=xpeT[k][:, (st0 + p_) * TCH:(st0 + p_ + 1) * TCH],
                                    rhs=w3t_sb[:, k, 256:384],
                                    start=(k == 0), stop=(k == 3),
                                )
                            # matmul cols = [v_h0 | v_h1] -> vsb dims 1, 3
                            nc.vector.tensor_copy(
                                out=vsb[:, st0 + p_, 1:4:2, :],
                                in_=ps[:, p_])
                    return go

                return [adds(), qk(0, qT), qk(1, kT),
                        vgrp(4 * w), vgrp(4 * w + 2)]

            def emit_outproj(i, atn, tail=False):
                def half(d0):
                    def go():
                        for d in (d0, d0 + 1):
                            op = psO.tile([128, SW], f32, tag="o")
                            if tail:
                                for hf in (slice(0, SW // 2),
                                           slice(SW // 2, SW)):
                                    nc.tensor.matmul(
                                        op[:, hf],
                                        lhsT=wct_sb[:, d * 128:(d + 1) * 128],
                                        rhs=atn[:, hf], start=True, stop=True,
                                    )
                            else:
                                nc.tensor.matmul(
                                    op, lhsT=wct_sb[:, d * 128:(d + 1) * 128],
                                    rhs=atn, start=True, stop=True,
                                )
                            ob = osp.tile([128, SW], bf16, tag="ob")
                            if tail:
                                nc.scalar.copy(out=ob, in_=op)
                            else:
                                nc.vector.tensor_copy(out=ob, in_=op)
                            nc.sync.dma_start(
                                out=outp[d * 128:(d + 1) * 128,
                                         i * SW:(i + 1) * SW],
                                in_=ob)
                    return go
                return [half(0), half(2)]

            # phase 2 of window 0 emitted up front; phase 2 of window 1
            # queued for weaving into window 0's attention loop
            for go in emit_phase2_slices(0):
                go()

            pending = list(emit_phase2_slices(1))  # weave queue, 2 pops/j-step

            for i in range(4):
                jmax = 4 * i + 3
                avs = [psV.tile([128, SW], f32, tag=f"av{h}", name=f"av{h}_{i}")
                       for h in range(HPC)]
                prev = None   # (j, D, pb2) awaiting AV emission (lag 1)
                for step, j in enumerate(range(jmax + 1)):
                    # D = first unmasked column of this (t-chunk, s-window):
                    # columns [0:D) are strictly-future for every row.
                    D = max(0, 128 * j - 512 * i)
                    sc2 = psS.tile([128, 2, SW], f32, tag="sc")
                    for h in range(HPC):
                        nc.tensor.matmul(
                            sc2[:, h, D:SW],
                            lhsT=kT[h * CC:(h + 1) * CC, j * TCH:(j + 1) * TCH],
                            rhs=qT[h * CC:(h + 1) * CC, i * SW + D:(i + 1) * SW],
                            start=True, stop=True,
                        )
                    pb2 = pbp.tile([128, 2, SW], bf16, tag="pb")
                    nc.scalar.activation(out=pb2[:, :, D:SW], in_=sc2[:, :, D:SW],
                                         func=Exp, scale=TEMP)
                    if j >= 4 * i:
                        # diagonal 128-col stripe: zero strictly-future
                        # entries; the last step's mask runs on DVE (faster)
                        # since it sits on the end-of-window critical chain
                        eng = nc.vector if j == jmax else nc.gpsimd
                        eng.tensor_mul(
                            out=pb2[:, :, D:D + 128],
                            in0=pb2[:, :, D:D + 128], in1=mask_sb)
                    if debug and i == 0 and j == 0:
                        nc.sync.dma_start(out=dpb[:, :, :], in_=pb2)
                    if prev is not None:
                        pj, pD, ppb = prev
                        for h in range(HPC):
                            nc.tensor.matmul(
                                avs[h][:, pD:SW],
                                lhsT=vsb[:, pj, 2 * h:2 * h + 2, :],
                                rhs=ppb[:, h, pD:SW],
                                start=(pj == 0), stop=False,
                            )
                    prev = (j, D, pb2)
                    if step == 0 and pending:
                        # one early pop: PE filler for the boundary DVE chain
                        pending.pop(0)()
                    steps_left = jmax + 1 - step
                    while pending and len(pending) >= steps_left:
                        pending.pop(0)()
                pj, pD, ppb = prev
                for h in range(HPC):
                    nc.tensor.matmul(
                        avs[h][:, pD:SW],
                        lhsT=vsb[:, pj, 2 * h:2 * h + 2, :],
                        rhs=ppb[:, h, pD:SW],
                        start=(pj == 0), stop=True,
                    )
                if debug and i == 0:
                    dav_sb = singles.tile([128, 2, SW], f32, tag="dav")
                    for h in range(HPC):
                        nc.vector.tensor_copy(out=dav_sb[:, h, :], in_=avs[h])
                    nc.sync.dma_start(out=dav[:, :, :], in_=dav_sb)
                # softmax normalization: den rows -> reciprocal -> scale values
                atn = atp.tile([128, SW], bf16, tag="atn")
                halves = (slice(0, SW),) if i < 3 else (
                    slice(0, SW // 2), slice(SW // 2, SW))
                for hf in halves:
                    for h in range(HPC):
                        # custom-DVE reciprocal needs base partition 0 on HW
                        rcb = rbp.tile([CC, SW], f32, tag="rcb")
                        nc.vector.reciprocal_approx_fast(
                            out=rcb[:, hf], in_=avs[h][0:CC, hf])
                        nc.vector.tensor_mul(
                            out=atn[h * CC:(h + 1) * CC, hf],
                            in0=avs[h][CC:2 * CC, hf], in1=rcb[:, hf],
                        )
                if debug:
                    nc.sync.dma_start(out=datn[:, i, :], in_=atn)
                ops = emit_outproj(i, atn, tail=(i == 3))
                if i < 2:
                    # weave order for window i+1: PE filler (adds+qk-q of
                    # window i+2) lands before outproj(i) so the atn chain
                    # has completed by the time PE reaches it
                    p2 = emit_phase2_slices(i + 2)
                    pending.extend([p2[0], p2[1], ops[0], ops[1]] + p2[2:])
                elif i == 2:
                    pending.extend(ops)
                else:
                    for go in pending:
                        go()
                    pending = []
                    for go in ops:
                        go()
            if debug:
                nc.sync.dma_start(out=dq[:, :], in_=qT)
                nc.sync.dma_start(out=dk[:, :], in_=kT)
                nc.sync.dma_start(out=dv[:, :, :, :], in_=vsb)

    nc.compile()
    return nc


def _get_nc():
    if "nc" not in _CACHE:
        _CACHE["nc"] = _build_bass()
    return _CACHE["nc"]


def _make_in_maps(x, pe, Wqkv, bqkv, Wc):
    bf = ml_dtypes.bfloat16
    tt = np.arange(128)[:, None]
    ss = np.arange(128)[None, :]
    m = (ss >= tt).astype(np.float32).astype(bf)          # keep where s' >= t'
    mask01 = np.ascontiguousarray(np.broadcast_to(m[:, None, :], (128, 2, 128)))

    xpe_b = {}
    for b in range(B):
        # [4w, 128p, 4k, 2, 512]: x/pe interleaved, chunk-major per window
        xw = x[:, b, :].T.reshape(4, 128, 4, SW).transpose(2, 1, 0, 3)
        pw = pe[:, b, :].T.reshape(4, 128, 4, SW).transpose(2, 1, 0, 3)
        xpe_b[b] = np.ascontiguousarray(np.stack([xw, pw], axis=3)).astype(bf)

    in_maps = []
    for core in range(NCORE):
        b, hg = core // 4, core % 4
        lo = hg * 128
        W3 = np.concatenate([Wqkv[lo:lo + 128],
                             Wqkv[C + lo:C + lo + 128],
                             Wqkv[2 * C + lo:2 * C + lo + 128]])   # (384, 512)
        w3t = np.ascontiguousarray(
            W3.T.reshape(4, 128, 384).transpose(1, 0, 2)).astype(bf)
        b3 = np.stack([bqkv[lo:lo + 128], bqkv[C + lo:C + lo + 128]], axis=1)
        b3 = np.ascontiguousarray(b3).astype(np.float32)
        wct = np.ascontiguousarray(Wc[:, lo:lo + 128].T).astype(bf)
        in_maps.append({
            "xpe": xpe_b[b], "w3t": w3t, "b3": b3,
            "wct": wct, "mask01": mask01,
        })
    return in_maps


def _numpy_fallback(x, pe, content_mask, Wqkv, bqkv, Wc, bc):
    xpe = (x + pe).astype(np.float32)
    qkv = xpe.reshape(-1, C) @ Wqkv.T + bqkv
    qkv = qkv.reshape(S, B, 3 * C)
    q, k, v = np.split(qkv, 3, axis=-1)
    q = q.reshape(S, B, H, CC)
    k = k.reshape(S, B, H, CC)
    v = v.reshape(S, B, H, CC)
    out = np.empty((S, B, C), np.float32)
    for b in range(B):
        for h in range(H):
            sc = (q[:, b, h] @ k[:, b, h].T) * np.float32(TEMP)
            sc = np.where(content_mask[:, :, b], -np.inf, sc)
            sc = sc - sc.max(axis=1, keepdims=True)
            p = np.exp(sc)
            p /= p.sum(axis=1, keepdims=True)
            out[:, b, h * CC:(h + 1) * CC] = p @ v[:, b, h]
    return (out.reshape(-1, C) @ Wc.T + bc).reshape(S, B, C).astype(np.float32)


def kernel(x, pe, content_mask, pad, Wqkv, bqkv, Wc, bc):
    x = np.asarray(x, dtype=np.float32)
    pe = np.asarray(pe, dtype=np.float32)
    content_mask = np.asarray(content_mask)
    Wqkv = np.asarray(Wqkv, dtype=np.float32)
    bqkv = np.asarray(bqkv, dtype=np.float32)
    Wc = np.asarray(Wc, dtype=np.float32)
    bc = np.asarray(bc, dtype=np.float32)

    idx = np.arange(S)
    causal = idx[None, :] > idx[:, None]
    if not np.array_equal(content_mask, np.broadcast_to(causal[:, :, None], (S, S, B))):
        return _numpy_fallback(x, pe, content_mask, Wqkv, bqkv, Wc, bc)

    from concourse.bass_utils import run_bass_kernel_spmd

    nc = _get_nc()
    in_maps = _make_in_maps(x, pe, Wqkv, bqkv, Wc)
    res = run_bass_kernel_spmd(nc, in_maps, core_ids=list(range(NCORE)))
    out = np.empty((S, B, C), np.float32)
    bc_eff = bc + Wc @ bqkv[2 * C:3 * C]   # v-bias folded through the output proj
    for b in range(B):
        acc = res.results[b * 4]["outp"].astype(np.float32).copy()
        for g in range(1, 4):
            acc += res.results[b * 4 + g]["outp"]
        out[:, b, :] = acc.T + bc_eff
    return out


# revision 22
# speedup vs baseline: 1.0184x; 1.0184x over previous
"""Trainium2 Bass kernel for nn_CompressedCausalAttention.

Sharding: 8 cores = 2 batches x 4 head-groups (2 heads each).
Per-core dataflow (chan-major "T" layouts are (channel partition, seq free)):
  phase 2 (per s-window, woven into the attention loop of the previous
           window): xpeT = x+pe (bf16 DVE add), qT/kT chan-major (bias via
           gpsimd), v seq-major into vsb chunks laid out
           [ones(64) | v_h0(64) | v_h1(64) | ones(64)] so each head's AV
           matmul replicates the softmax denominator into 64 PSUM rows for
           free.
  phase 3: flash attention, scores (t-part, s-free), no max subtraction
           (scores bounded), causal mask = 0/1 multiply on gpsimd after exp,
           denominator inverted with reciprocal_approx_fast.
  phase 4: partial output projection outpT = Wc_mine^T-slice @ attnTn, woven
           into the next window's attention loop.
Host: shards inputs (incl. transposes, bf16 cast), sums the 4 per-batch
partials, adds bc (v-bias folded through the output projection).
"""

import numpy as np
import ml_dtypes

S, B, C, H = 2048, 2, 512, 8
CC = C // H            # 64
HPC = 2                # heads per core
NCORE = 8
SW = 512               # s window (free dim of score tiles)
TCH = 128              # t chunk (partition dim of score tiles)
TEMP = 1.0 / 8.0       # 1/sqrt(CC)

_CACHE = {}


def _build_bass(debug=False):
    import concourse.bass as bass
    import concourse.mybir as mybir
    import concourse.tile as tile
    from concourse import bacc

    f32 = mybir.dt.float32
    bf16 = mybir.dt.bfloat16

    nc = bacc.Bacc("TRN2", target_bir_lowering=False)
    xpe_d = nc.declare_dram_parameter("xpe", [4, 128, 4, 2, SW], bf16, isOutput=False)
    w3t = nc.declare_dram_parameter("w3t", [128, 4, 384], bf16, isOutput=False)
    b3 = nc.declare_dram_parameter("b3", [128, 2], f32, isOutput=False)
    wct = nc.declare_dram_parameter("wct", [128, C], bf16, isOutput=False)
    mask01 = nc.declare_dram_parameter("mask01", [128, 2, 128], bf16, isOutput=False)
    outp = nc.declare_dram_parameter("outp", [C, S], bf16, isOutput=True)
    if debug:
        dq = nc.declare_dram_parameter("dq", [128, S], bf16, isOutput=True)
        dk = nc.declare_dram_parameter("dk", [128, S], bf16, isOutput=True)
        dv = nc.declare_dram_parameter("dv", [128, 16, 4, CC], bf16, isOutput=True)
        dpb = nc.declare_dram_parameter("dpb", [128, 2, SW], bf16, isOutput=True)
        dav = nc.declare_dram_parameter("dav", [128, 2, SW], f32, isOutput=True)
        datn = nc.declare_dram_parameter("datn", [128, 4, SW], bf16, isOutput=True)

    Exp = mybir.ActivationFunctionType.Exp

    with tile.TileContext(nc) as tc:
        with (
            tc.tile_pool(name="singles", bufs=1) as singles,
            tc.tile_pool(name="pbp", bufs=6) as pbp,
            tc.tile_pool(name="atp", bufs=2) as atp,
            tc.tile_pool(name="rbp", bufs=2) as rbp,
            tc.tile_pool(name="osp", bufs=4) as osp,
            tc.tile_pool(name="psS", bufs=2, space="PSUM") as psS,
            tc.tile_pool(name="psV", bufs=1, space="PSUM") as psV,
            tc.tile_pool(name="psO", bufs=2, space="PSUM") as psO,
        ):
            # ---- constants + window-0/1 inputs (DMA'd first) ----
            w3t_sb = singles.tile([128, 4, 384], bf16, tag="w3t")
            nc.sync.dma_start(out=w3t_sb, in_=w3t[:, :, :])
            # x/pe interleaved per window: [128, w, k, {x,pe}, s']
            xpw = singles.tile([128, 4, 4, 2, SW], bf16, tag="xpw")
            for k in range(4):
                nc.sync.dma_start(out=xpw[:, 0, k], in_=xpe_d[0, :, k])
            b3_sb = singles.tile([128, 2], f32, tag="b3")
            nc.sync.dma_start(out=b3_sb, in_=b3[:, :])
            nc.sync.dma_start(out=xpw[:, 1], in_=xpe_d[1])
            wct_sb = singles.tile([128, C], bf16, tag="wct")
            nc.sync.dma_start(out=wct_sb, in_=wct[:, :])
            mask_sb = singles.tile([128, 2, 128], bf16, tag="mask")
            nc.sync.dma_start(out=mask_sb, in_=mask01[:, :, :])

            xpeT = [singles.tile([128, S], bf16, tag=f"xpeT{k}", name=f"xpeT{k}")
                    for k in range(4)]
            qT = singles.tile([128, S], bf16, tag="qT")
            kT = singles.tile([128, S], bf16, tag="kT")
            # v seq-major: chunk layout [ones | v_h0 | ones | v_h1]; head h's
            # AV stationary is cols [128*h : 128*h+128] = [ones | v_h] -> AV
            # output rows 0:CC hold the softmax denominator replicated (base
            # partition 0, required by the custom-DVE reciprocal), rows
            # CC:2CC hold the values.
            vsb = singles.tile([128, 16, 4, CC], bf16, tag="vsb")
            nc.vector.memset(vsb[:, :, 0, :], 1.0)
            nc.vector.memset(vsb[:, :, 2, :], 1.0)

            def emit_phase2_slices(w):
                """Emit-closures for phase-2 of window w (q/k/v projections)."""
                sl = slice(w * SW, (w + 1) * SW)

                def adds():
                    def go():
                        if 1 <= w < 3:
                            # this slice runs during window w-1; kick off the
                            # DMA for window w+1 (w0/w1 are loaded up front)
                            nc.sync.dma_start(out=xpw[:, w + 1],
                                              in_=xpe_d[w + 1])
                        eng = nc.vector if w == 0 else nc.gpsimd
                        for k in range(4):
                            eng.tensor_add(
                                out=xpeT[k][:, sl],
                                in0=xpw[:, w, k, 0, :], in1=xpw[:, w, k, 1, :])
                    return go

                def qk(blk, dst):
                    def go():
                        ps = psO.tile([128, SW], f32, tag="o")
                        for k in range(4):
                            nc.tensor.matmul(
                                ps,
                                lhsT=w3t_sb[:, k, blk * 128:(blk + 1) * 128],
                                rhs=xpeT[k][:, sl],
                                start=(k == 0), stop=(k == 3),
                            )
                        nc.vector.tensor_scalar_add(
                            out=dst[:, sl], in0=ps,
                            scalar1=b3_sb[:, blk:blk + 1])
                    return go

                def vgrp(st0):
                    def go():
                        ps = psO.tile([128, 2, 2, CC], f32, tag="o")
                        for p_ in range(2):
                            for k in range(4):
                                nc.tensor.matmul(
                                    ps[:, p_],
                                    lhsT=xpeT[k][:, (st0 + p_) * TCH:(st0 + p_ + 1) * TCH],
                                    rhs=w3t_sb[:, k, 256:384],
                                    start=(k == 0), stop=(k == 3),
                                )
                            # matmul cols = [v_h0 | v_h1] -> vsb dims 1, 3
                            nc.vector.tensor_copy(
                                out=vsb[:, st0 + p_, 1:4:2, :],
                                in_=ps[:, p_])
                    return go

                return [adds(), qk(0, qT), qk(1, kT),
                        vgrp(4 * w), vgrp(4 * w + 2)]

            def emit_outproj(i, atn, tail=False):
                def half(d0):
                    def go():
                        for d in (d0, d0 + 1):
                            op = psO.tile([128, SW], f32, tag="o")
                            if tail:
                                for hf in (slice(0, SW // 2),
                                           slice(SW // 2, SW)):
                                    nc.tensor.matmul(
                                        op[:, hf],
                                        lhsT=wct_sb[:, d * 128:(d + 1) * 128],
                                        rhs=atn[:, hf], start=True, stop=True,
                                    )
                            else:
                                nc.tensor.matmul(
                                    op, lhsT=wct_sb[:, d * 128:(d + 1) * 128],
                                    rhs=atn, start=True, stop=True,
                                )
                            ob = osp.tile([128, SW], bf16, tag="ob")
                            if tail:
                                nc.scalar.copy(out=ob, in_=op)
                            else:
                                nc.vector.tensor_copy(out=ob, in_=op)
                            nc.sync.dma_start(
                                out=outp[d * 128:(d + 1) * 128,
                                         i * SW:(i + 1) * SW],
                                in_=ob)
                    return go
                return [half(0), half(2)]

            # phase 2 of window 0 emitted up front; phase 2 of window 1
            # queued for weaving into window 0's attention loop
            for go in emit_phase2_slices(0):
                go()

            pending = list(emit_phase2_slices(1))  # weave queue
            preroll = []   # (j, D, pb2) of the next window, scored early

            def emit_score_step(i, j, diag_eng=None):
                """Scores+exp (+mask) for (i, j); returns (j, D, pb2)."""
                D = max(0, 128 * j - 512 * i)
                sc2 = psS.tile([128, 2, SW], f32, tag="sc")
                for h in range(HPC):
                    nc.tensor.matmul(
                        sc2[:, h, D:SW],
                        lhsT=kT[h * CC:(h + 1) * CC, j * TCH:(j + 1) * TCH],
                        rhs=qT[h * CC:(h + 1) * CC, i * SW + D:(i + 1) * SW],
                        start=True, stop=True,
                    )
                pb2 = pbp.tile([128, 2, SW], bf16, tag="pb")
                nc.scalar.activation(out=pb2[:, :, D:SW], in_=sc2[:, :, D:SW],
                                     func=Exp, scale=TEMP)
                if j >= 4 * i:
                    # diagonal 128-col stripe: zero strictly-future entries
                    diag_eng.tensor_mul(
                        out=pb2[:, :, D:D + 128],
                        in0=pb2[:, :, D:D + 128], in1=mask_sb)
                return (j, D, pb2)

            def emit_av(i, avs, entry, stop):
                pj, pD, ppb = entry
                for h in range(HPC):
                    nc.tensor.matmul(
                        avs[h][:, pD:SW],
                        lhsT=vsb[:, pj, 2 * h:2 * h + 2, :],
                        rhs=ppb[:, h, pD:SW],
                        start=(pj == 0), stop=stop,
                    )

            for i in range(4):
                jmax = 4 * i + 3
                avs = [psV.tile([128, SW], f32, tag=f"av{h}", name=f"av{h}_{i}")
                       for h in range(HPC)]
                scored = list(preroll)   # scored but not yet AV'd (FIFO)
                preroll = []
                for step, j in enumerate(range(len(scored), jmax + 1)):
                    # the last diag step's mask runs on DVE (faster) since it
                    # sits on the end-of-window critical chain
                    scored.append(emit_score_step(
                        i, j, nc.vector if j == jmax else nc.gpsimd))
                    while len(scored) > 1:
                        emit_av(i, avs, scored.pop(0), stop=False)
                    if step == 0 and pending:
                        # one early pop: PE filler for the boundary DVE chain
                        pending.pop(0)()
                    steps_left = jmax - j + 1
                    while pending and len(pending) >= steps_left:
                        pending.pop(0)()
                # pre-roll: score the first 2 steps of the next window before
                # flushing this window's final AV -> PE filler for the
                # final-exp wait and the boundary DVE chain
                if i < 3:
                    for jn in range(2):
                        preroll.append(emit_score_step(i + 1, jn, nc.gpsimd))
                while scored:
                    emit_av(i, avs, scored.pop(0), stop=(not scored))
                if debug and i == 0:
                    dav_sb = singles.tile([128, 2, SW], f32, tag="dav")
                    for h in range(HPC):
                        nc.vector.tensor_copy(out=dav_sb[:, h, :], in_=avs[h])
                    nc.sync.dma_start(out=dav[:, :, :], in_=dav_sb)
                # softmax normalization: den rows -> reciprocal -> scale values
                atn = atp.tile([128, SW], bf16, tag="atn")
                halves = (slice(0, SW),) if i < 3 else (
                    slice(0, SW // 2), slice(SW // 2, SW))
                for hf in halves:
                    for h in range(HPC):
                        # custom-DVE reciprocal needs base partition 0 on HW
                        rcb = rbp.tile([CC, SW], f32, tag="rcb")
                        nc.vector.reciprocal_approx_fast(
                            out=rcb[:, hf], in_=avs[h][0:CC, hf])
                        nc.vector.tensor_mul(
                            out=atn[h * CC:(h + 1) * CC, hf],
                            in0=avs[h][CC:2 * CC, hf], in1=rcb[:, hf],
                        )
                if debug:
                    nc.sync.dma_start(out=datn[:, i, :], in_=atn)
                ops = emit_outproj(i, atn, tail=(i == 3))
                if i < 2:
                    # weave order for window i+1: PE filler (adds+qk-q of
                    # window i+2) lands before outproj(i) so the atn chain
                    # has completed by the time PE reaches it
                    p2 = emit_phase2_slices(i + 2)
                    pending.extend([p2[0], p2[1], ops[0], ops[1]] + p2[2:])
                elif i == 2:
                    pending.extend(ops)
                else:
                    for go in pending:
                        go()
                    pending = []
                    for go in ops:
                        go()
            if debug:
                nc.sync.dma_start(out=dq[:, :], in_=qT)
                nc.sync.dma_start(out=dk[:, :], in_=kT)
                nc.sync.dma_start(out=dv[:, :, :, :], in_=vsb)

    nc.compile()
    return nc


def _get_nc():
    if "nc" not in _CACHE:
        _CACHE["nc"] = _build_bass()
    return _CACHE["nc"]


def _make_in_maps(x, pe, Wqkv, bqkv, Wc):
    bf = ml_dtypes.bfloat16
    tt = np.arange(128)[:, None]
    ss = np.arange(128)[None, :]
    m = (ss >= tt).astype(np.float32).astype(bf)          # keep where s' >= t'
    mask01 = np.ascontiguousarray(np.broadcast_to(m[:, None, :], (128, 2, 128)))

    xpe_b = {}
    for b in range(B):
        # [4w, 128p, 4k, 2, 512]: x/pe interleaved, chunk-major per window
        xw = x[:, b, :].T.reshape(4, 128, 4, SW).transpose(2, 1, 0, 3)
        pw = pe[:, b, :].T.reshape(4, 128, 4, SW).transpose(2, 1, 0, 3)
        xpe_b[b] = np.ascontiguousarray(np.stack([xw, pw], axis=3)).astype(bf)

    in_maps = []
    for core in range(NCORE):
        b, hg = core // 4, core % 4
        lo = hg * 128
        W3 = np.concatenate([Wqkv[lo:lo + 128],
                             Wqkv[C + lo:C + lo + 128],
                             Wqkv[2 * C + lo:2 * C + lo + 128]])   # (384, 512)
        w3t = np.ascontiguousarray(
            W3.T.reshape(4, 128, 384).transpose(1, 0, 2)).astype(bf)
        b3 = np.stack([bqkv[lo:lo + 128], bqkv[C + lo:C + lo + 128]], axis=1)
        b3 = np.ascontiguousarray(b3).astype(np.float32)
        wct = np.ascontiguousarray(Wc[:, lo:lo + 128].T).astype(bf)
        in_maps.append({
            "xpe": xpe_b[b], "w3t": w3t, "b3": b3,
            "wct": wct, "mask01": mask01,
        })
    return in_maps


def _numpy_fallback(x, pe, content_mask, Wqkv, bqkv, Wc, bc):
    xpe = (x + pe).astype(np.float32)
    qkv = xpe.reshape(-1, C) @ Wqkv.T + bqkv
    qkv = qkv.reshape(S, B, 3 * C)
    q, k, v = np.split(qkv, 3, axis=-1)
    q = q.reshape(S, B, H, CC)
    k = k.reshape(S, B, H, CC)
    v = v.reshape(S, B, H, CC)
    out = np.empty((S, B, C), np.float32)
    for b in range(B):
        for h in range(H):
            sc = (q[:, b, h] @ k[:, b, h].T) * np.float32(TEMP)
            sc = np.where(content_mask[:, :, b], -np.inf, sc)
            sc = sc - sc.max(axis=1, keepdims=True)
            p = np.exp(sc)
            p /= p.sum(axis=1, keepdims=True)
            out[:, b, h * CC:(h + 1) * CC] = p @ v[:, b, h]
    return (out.reshape(-1, C) @ Wc.T + bc).reshape(S, B, C).astype(np.float32)


def kernel(x, pe, content_mask, pad, Wqkv, bqkv, Wc, bc):
    x = np.asarray(x, dtype=np.float32)
    pe = np.asarray(pe, dtype=np.float32)
    content_mask = np.asarray(content_mask)
    Wqkv = np.asarray(Wqkv, dtype=np.float32)
    bqkv = np.asarray(bqkv, dtype=np.float32)
    Wc = np.asarray(Wc, dtype=np.float32)
    bc = np.asarray(bc, dtype=np.float32)

    idx = np.arange(S)
    causal = idx[None, :] > idx[:, None]
    if not np.array_equal(content_mask, np.broadcast_to(causal[:, :, None], (S, S, B))):
        return _numpy_fallback(x, pe, content_mask, Wqkv, bqkv, Wc, bc)

    from concourse.bass_utils import run_bass_kernel_spmd

    nc = _get_nc()
    in_maps = _make_in_maps(x, pe, Wqkv, bqkv, Wc)
    res = run_bass_kernel_spmd(nc, in_maps, core_ids=list(range(NCORE)))
    out = np.empty((S, B, C), np.float32)
    bc_eff = bc + Wc @ bqkv[2 * C:3 * C]   # v-bias folded through the output proj
    for b in range(B):
        acc = res.results[b * 4]["outp"].astype(np.float32).copy()
        for g in range(1, 4):
            acc += res.results[b * 4 + g]["outp"]
        out[:, b, :] = acc.T + bc_eff
    return out
